# revision 5
# baseline (speedup 1.0000x reference)
"""Trainium2 Bass kernel for nn_NeuronS3DiffUpsample2D.

Reference computation (per sample b):
    up   = nearest-2x-upsample(x[b])                       # [C, 320, 320]
    w    = Wb + 0.25 * einsum('or,rikl->oikl', lora_up, lora_down)
    w_b  = w * de_mod[b, None, :, None, None]              # modulate input chans
    dem  = rsqrt(sum_{i,k,l} w_b^2 + eps)                  # per output chan
    y[b] = conv2d(up, w_b, SAME) * dem + bias

Key algebraic transform: a 3x3 SAME conv on a 2x nearest-upsampled image
decomposes into 4 output phases (di, dj in {0,1}), each a 2x2 conv on the
ORIGINAL 160x160 input:
    y[2i+di, 2j+dj] = sum_{a,b in {0,1}} K[di,dj,a,b] @ x[i+a+di-1, j+b+dj-1]
where the 16 [O, I] matrices K are sums of 1/2/4 of the 9 taps of w.
This is 4/9 of the naive FLOPs and never materializes the upsampled image.
The demod scale is per output channel and conv is linear in w, so the conv
OUTPUT is scaled by dem[o] at PSUM eviction, fused with the bias add.

All tensors ride bf16 (rel-err budget is 2e-2; bf16 end-to-end lands ~4e-3):
x is host-padded to [C,162,162] bf16 so every band DMA is one contiguous
descriptor per partition (no per-row 640B descriptors, no on-device border
zeroing), matmuls are bf16 (1 cycle/row at any free size - also kills the
f32r N<256 4x penalty on the R=1 tail block), and y is written bf16 and
upcast on the host.  Total HBM traffic drops from 66 MB to ~35 MB per core,
taking DMA well below the PE's ~173 us of matmul streaming.

The demod reduction runs in a second [o, (t,i)] weight layout so the
partition-axis sum becomes a free-axis reduce: 3 small early matmuls build
delta in [o,.] form, then gpsimd/DVE/ACT finish rsqrt off the PE's program
order (the baseline's ones-matmul made the PE wait ~10us on a DVE chain).
Dummy "warm" matmuls keep the PE busy through the weight stage so the
2.4 GHz pstate ramp completes before the main conv stream starts.

Sharding: data-parallel over batch B=8 across 8 NeuronCores; each core
builds its own per-sample weights locally.  Host-side work is layout only:
padding, per-sample slicing/replication, transposes, dtype casts.
"""

import os
import sys
import numpy as np
from contextlib import ExitStack

_NOWARM = bool(int(os.environ.get("K_NOWARM", "0")))
_NOGPS = bool(int(os.environ.get("K_NOGPS", "0")))
_NODEMO = bool(int(os.environ.get("K_NODEMO", "0")))   # demod=1.0 (wrong numerics, hang-test only)
_F32OUT = bool(int(os.environ.get("K_F32OUT", "0")))
_NOACTMUL = bool(int(os.environ.get("K_NOACTMUL", "0")))
_DEBUG = bool(int(os.environ.get("K_DEBUG", "0")))

try:
    import concourse.bass as bass
except ImportError:  # grading env without the axon PYTHONPATH
    sys.path.insert(0, "/opt/trn_rl_repo")
    import concourse.bass as bass
import ml_dtypes
import concourse.tile as tile
from concourse import bacc, mybir
from concourse.bass_utils import run_bass_kernel_spmd

B, C, H, W = 8, 128, 160, 160
RANK = 32
SCALING = 0.25
EPS = 1e-8
HP, WP = H + 2, W + 2      # host-padded input (zero border baked in)
R_BLK = 3                  # x-rows per matmul block -> N = 3*160 = 480 <= 512
# x-row band split: band 0 tiny so the first matmuls gate only on ~0.3 MB.
BAND_ROWS = [3, 12, 27, 27, 27, 27, 27, 10]
BAND_START = [0, 3, 15, 42, 69, 96, 123, 150]
NBANDS = len(BAND_ROWS)
NBLOCKS = 54               # 53 blocks of 3 rows + 1 tail row
N_WARM = 6                 # PE pstate prewarm matmuls (memset-gated, pre-LL)
NCORES = 8

f32 = mybir.dt.float32
bf16 = mybir.dt.bfloat16


def _conv_kernel(ctx, tc, y, x, wbT, ll_d, idm_d, dbg=None):
    nc = tc.nc
    AF = mybir.ActivationFunctionType
    ALU = mybir.AluOpType
    AX = mybir.AxisListType

    const = ctx.enter_context(tc.tile_pool(name="const", bufs=1))
    bands = ctx.enter_context(tc.tile_pool(name="bands", bufs=4))

    comb = const.tile([128, 16, C], bf16)        # 16 combined taps, [i, slot, o]
    demP = const.tile([128, 1], f32)             # rsqrt demod, per output chan
    biasT = const.tile([128, 1], f32)            # bias, copied out of IDm

    # ---- input bands: host-padded rows [S, S+rows+2) land contiguously; one
    # descriptor per partition.  Bands 0/1 ride the sync HWDGE ring (boots
    # first), the rest the gpsimd SWDGE ring.
    band_tiles = []
    band_dmas = []
    for bb in range(NBANDS):
        n = BAND_ROWS[bb] + 2
        bt = bands.tile([128, n, WP], bf16, tag="band", name=f"band{bb}")
        band_tiles.append(bt)
        band_dmas.append((bt[:, 0:n, :], x[:, BAND_START[bb] : BAND_START[bb] + n, :]))

    with tc.tile_pool(name="wtmp", bufs=1) as wtmp, tc.tile_pool(
        name="wpsum", bufs=1, space="PSUM"
    ) as wpsum:
        # warm-matmul operand: a memset tile so the PE can start the moment
        # the engines boot (~6.4us) instead of gating on the LL DMA (~10us).
        # The DVFS ramp to 2.4 GHz takes ~15us of sustained PE activity, so
        # every us of earlier PE onset is a us of main-loop time at full
        # clock instead of 1.2 GHz.
        warmT = wtmp.tile([128, 512], bf16, name="warmsrc")
        nc.gpsimd.memset(warmT[:], 0.0)

        LL = wtmp.tile([RANK, 1280], bf16)       # lora_down^T (t,i) | 0.25*lora_up^T
        WbTS = wtmp.tile([128, 9, C], bf16)      # Wb^T: [i, t, o]
        IDm = wtmp.tile([128, 66], f32)          # de_mod,bias | bf16 eye
        LD9 = LL[:, 0:1152].rearrange("p (t c) -> p t c", c=C)
        LUTn = LL[:, 1152:1280]
        dmb = IDm[:, 0:2]
        identb = IDm[:, 2:66].bitcast(bf16)

        # The SWDGE (gpsimd) queue hits full rate (~380 GB/s) immediately,
        # while the HWDGE (sync) queue crawls at <70 GB/s for its first ~8us.
        # So ALL startup tensors ride SWDGE, gate tensors first; sync gets a
        # small primer so its ramp is done before the first output eviction.
        nc.gpsimd.dma_start(LL[:], ll_d[:])
        nc.gpsimd.dma_start(IDm[:], idm_d[:])
        nc.gpsimd.dma_start(WbTS[:], wbT[:])
        for bb in range(4):
            nc.gpsimd.dma_start(*band_dmas[bb])
        LLscratch = wtmp.tile([RANK, 1280], bf16, name="llscratch")
        nc.sync.dma_start(LLscratch[:], ll_d[:])

        # ---- PE early: prewarm on the memset tile (no DMA dependency),
        # then per tap an adjacent lora+identity pair accumulating
        #   deltaP[i,t,o] = sum_r down[r,i,t]*0.25*up[o,r] + Wb^T[i,t,o]
        # (folds the base weight in on the PE; wm3 is then a single de_mod
        # multiply on DVE).  The pairs must be adjacent: interleaving other
        # matmuls inside an open accumulation group drops the first write.
        if not _NOWARM:
            warmP = wpsum.tile([128, 384], f32)
            for _ in range(N_WARM):
                nc.tensor.matmul(
                    warmP[:], warmT[:, 0:128], warmT[:, 128:512],
                    start=True, stop=True,
                )
        deltaP = wpsum.tile([128, 9, C], f32)
        gps = nc.vector if _NOGPS else nc.gpsimd
        wm3 = wtmp.tile([128, 9, C], bf16)
        # Per-tap pipeline: the DVE de_mod multiply for tap t chases the PE
        # pair for tap t, so wm3 (and everything downstream: transposes,
        # demod squares, comb slots) is ready ~0.2us after the LAST pair
        # instead of gating on a monolithic [128,1152] DVE op.
        for t in range(9):
            nc.tensor.matmul(
                deltaP[:, t, :], LD9[:, t, :], LUTn[:], start=True, stop=False
            )
            nc.tensor.matmul(
                deltaP[:, t, :], identb[:], WbTS[:, t, :], start=False, stop=True
            )
        for t in range(9):
            nc.vector.tensor_scalar_mul(wm3[:, t, :], deltaP[:, t, :], dmb[:, 0:1])

        # bias is read during evictions long after IDm's SBUF may recycle
        nc.scalar.copy(biasT[:], dmb[:, 1:2])

        # 16 combined tap matrices.  Row combos over ki (t = 3*ki + kj):
        #   (di=0, a=0): ki0        (di=0, a=1): ki1+ki2
        #   (di=1, a=0): ki0+ki1    (di=1, a=1): ki2
        # and the same pattern over kj for (dj, b).
        R01 = wtmp.tile([128, 3, C], bf16)
        nc.vector.tensor_add(R01[:], wm3[:, 3:6, :], wm3[:, 6:9, :])
        R10 = wtmp.tile([128, 3, C], bf16)
        gps.tensor_add(R10[:], wm3[:, 0:3, :], wm3[:, 3:6, :])
        rowsrc = {
            (0, 0): wm3[:, 0:3, :],
            (0, 1): R01[:],
            (1, 0): R10[:],
            (1, 1): wm3[:, 6:9, :],
        }

        # comb slot layout: slot = 8*di + 2*a + 4*dj + b (bf16 on write).
        # DVE builds phases 0/2, gpsimd phases 1/3, in PE-use order.
        def build_slot(eng, p, q):
            di, dj = p >> 1, p & 1
            a, b = q >> 1, q & 1
            S = rowsrc[(di, a)]
            dst = comb[:, 8 * di + 2 * a + 4 * dj + b, :]
            if dj == 0 and b == 0:
                eng.tensor_copy(dst, S[:, 0, :])
            elif dj == 1 and b == 1:
                eng.tensor_copy(dst, S[:, 2, :])
            elif dj == 0:
                eng.tensor_add(dst, S[:, 1, :], S[:, 2, :])
            else:
                eng.tensor_add(dst, S[:, 0, :], S[:, 1, :])

        for q in range(4):
            build_slot(nc.vector, 0, q)
        for q in range(4):
            build_slot(gps, 1, q)
        for q in range(4):
            build_slot(gps, 3, q)
        for q in range(4):
            build_slot(nc.vector, 2, q)

        # ---- demod: PE-transpose wm3 into [o, (t,i)] PSUM, then ACT Square
        # passes whose accum_out IS the free-axis sum.  No extra DMAs, no
        # partition reduction.  The Square is chunked 3 taps at a time so it
        # chases the transposes: the PSUM banks (deltaP/wmOT) are fully read
        # ~1us after the last transpose, unblocking the main loop's first
        # mpsum tiles (which reuse this PSUM space) that much earlier.
        if not _NODEMO:
            wmOT = wpsum.tile([128, 9, C], bf16)
            for t in range(9):
                nc.tensor.transpose(wmOT[:, t, :], wm3[:, t, :], identb[:])
        if _NODEMO:
            nc.vector.memset(demP[:], 1.0)
        else:
            sqscr = wtmp.tile([128, 9, C], f32)
            s2c = wtmp.tile([128, 3], f32)
            for cch in range(3):
                nc.scalar.activation(
                    sqscr[:, 3 * cch : 3 * cch + 3, :],
                    wmOT[:, 3 * cch : 3 * cch + 3, :],
                    AF.Square,
                    accum_out=s2c[:, cch : cch + 1],
                )
            s2o = wtmp.tile([128, 1], f32)
            nc.vector.tensor_add(s2o[:], s2c[:, 0:1], s2c[:, 1:2])
            t1 = wtmp.tile([128, 1], f32)
            nc.vector.tensor_add(t1[:], s2o[:], s2c[:, 2:3])
            t1b = wtmp.tile([128, 1], f32)
            nc.vector.tensor_scalar_add(t1b[:], t1[:], EPS)
            t2 = wtmp.tile([128, 1], f32)
            nc.scalar.sqrt(t2[:], t1b[:])
            nc.vector.reciprocal(demP[:], t2[:])

        if dbg is not None:
            nc.sync.dma_start(dbg["wm3"][:], wm3[:])
            nc.sync.dma_start(dbg["comb"][:], comb.rearrange("p s c -> p (s c)"))
            nc.sync.dma_start(dbg["demP"][:], demP[:])

        # now it is safe to park gpsimd on the recycled-band WAR semaphores
        # (bands 4+ reuse band 0-3 buffers)
        for bb in range(4, NBANDS):
            nc.gpsimd.dma_start(*band_dmas[bb])

    # ---- main conv loop ----
    mpsum = ctx.enter_context(tc.tile_pool(name="mpsum", bufs=8, space="PSUM"))
    opool = ctx.enter_context(tc.tile_pool(name="obuf", bufs=3))

    # blocks of 3 x-rows; pairs of blocks share one output staging tile so
    # the sync sequencer issues half as many (128-descriptor) output DMAs.
    blk_band = []
    for g in range(NBLOCKS):
        i0 = g * R_BLK
        bb = next(
            b for b in range(NBANDS)
            if BAND_START[b] <= i0 and i0 + min(R_BLK, H - i0) <= BAND_START[b] + BAND_ROWS[b]
        )
        blk_band.append(bb)

    ob = None
    for g in range(NBLOCKS):
        i0 = g * R_BLK
        R = min(R_BLK, H - i0)
        bb = blk_band[g]
        bt = band_tiles[bb]
        ph = []
        for p in range(4):
            di, dj = p >> 1, p & 1
            pt = mpsum.tile([128, R * W], f32, tag="ph", name=f"ph{p}_{i0}")
            for q in range(4):
                a, b = q >> 1, q & 1
                tr = i0 + a + di - BAND_START[bb]     # padded-tile row
                co = b + dj                           # padded-tile col
                rhs = bt[:, tr : tr + R, co : co + W]
                slot = 8 * di + 2 * a + 4 * dj + b
                nc.tensor.matmul(
                    pt[:], comb[:, slot, :], rhs,
                    start=(q == 0), stop=(q == 3),
                )
            ph.append(pt)

        if g % 2 == 0:
            RR = R + (min(R_BLK, H - i0 - R) if g + 1 < NBLOCKS else 0)
            ob = opool.tile(
                [128, RR, 2, 2 * W], f32 if _F32OUT else bf16, tag="ob", name=f"ob_{i0}"
            )
            lr0 = 0
        else:
            lr0 = R_BLK
        obv = ob.rearrange("p r d (j two) -> p r d two j", two=2)
        # interleave phases into full output rows; scale by demod, add bias
        for p in range(4):
            di, dj = p >> 1, p & 1
            dst = obv[:, lr0 : lr0 + R, di, dj, :]
            srcv = ph[p].rearrange("p (r j) -> p r j", r=R)
            if dj == 0:
                nc.vector.tensor_scalar(
                    dst, srcv, demP[:, 0:1], biasT[:, 0:1],
                    op0=ALU.mult, op1=ALU.add,
                )
            else:
                nc.scalar.activation(
                    dst, srcv, AF.Identity, bias=biasT[:, 0:1], scale=demP[:, 0:1]
                )
        if g % 2 == 1 or g == NBLOCKS - 1:
            g0 = g - (g % 2)
            y0 = 2 * g0 * R_BLK
            nrows = 2 * ob.shape[1]
            if g == NBLOCKS - 1:
                # final group: split across BOTH DMA rings so the end-of-
                # kernel drain (which gates the teardown barrier) halves.
                rh = ob.shape[1] // 2
                nc.sync.dma_start(y[:, y0 : y0 + 2 * rh, :], ob[:, 0:rh])
                nc.gpsimd.dma_start(y[:, y0 + 2 * rh : y0 + nrows, :], ob[:, rh:])
            else:
                eng = nc.sync if (g0 // 2) % 2 == 0 else nc.gpsimd
                eng.dma_start(y[:, y0 : y0 + nrows, :], ob[:])


def _build():
    nc = bacc.Bacc(
        "TRN2",
        target_bir_lowering=False,
        debug=False,
        enable_asserts=False,
        num_devices=NCORES,
    )
    x = nc.dram_tensor("x", [C, HP, WP], bf16, kind="ExternalInput").ap()
    wbT = nc.dram_tensor("WbT", [C, 9 * C], bf16, kind="ExternalInput").ap()
    ll_d = nc.dram_tensor("LL", [RANK, 1280], bf16, kind="ExternalInput").ap()
    idm_d = nc.dram_tensor("IDm", [128, 66], f32, kind="ExternalInput").ap()
    y = nc.dram_tensor(
        "y", [C, 2 * H, 2 * W], f32 if _F32OUT else bf16, kind="ExternalOutput"
    ).ap()
    dbg = None
    if _DEBUG:
        dbg = {
            "wm3": nc.dram_tensor("dbg_wm3", [C, 9 * C], f32, kind="ExternalOutput").ap(),
            "comb": nc.dram_tensor("dbg_comb", [C, 16 * C], bf16, kind="ExternalOutput").ap(),
            "demP": nc.dram_tensor("dbg_demP", [C, 1], f32, kind="ExternalOutput").ap(),
        }

    with tile.TileContext(nc) as tc:
        with ExitStack() as ctx:
            _conv_kernel(ctx, tc, y, x, wbT, ll_d, idm_d, dbg)
    nc.compile()
    return nc


_CACHE = {}


def _get_nc():
    if "nc" not in _CACHE:
        _CACHE["nc"] = _build()
    return _CACHE["nc"]


def _make_in_maps(x, de_mod, Wb, lora_up, lora_down, bias):
    x = np.asarray(x, dtype=np.float32)
    de_mod = np.asarray(de_mod, dtype=np.float32)
    Wb = np.asarray(Wb, dtype=np.float32)
    lora_up = np.asarray(lora_up, dtype=np.float32)
    lora_down = np.asarray(lora_down, dtype=np.float32)
    bias = np.asarray(bias, dtype=np.float32).reshape(C)
    # layout-only host prep: pad + cast x, transpose/replicate weights
    xp = np.zeros((B, C, HP, WP), dtype=ml_dtypes.bfloat16)
    xp[:, :, 1 : 1 + H, 1 : 1 + W] = x.astype(ml_dtypes.bfloat16)
    wbT = np.ascontiguousarray(
        Wb.transpose(1, 2, 3, 0).reshape(C, 9 * C)
    ).astype(ml_dtypes.bfloat16)
    luT = np.ascontiguousarray((SCALING * lora_up).T).astype(ml_dtypes.bfloat16)
    ldT = np.ascontiguousarray(
        lora_down.transpose(0, 2, 3, 1).reshape(RANK, 9 * C)
    ).astype(ml_dtypes.bfloat16)
    ll = np.concatenate([ldT, luT], axis=1)      # [32, 1280] bf16
    eye_b = np.eye(128, dtype=ml_dtypes.bfloat16)
    eye_b_as_f = np.ascontiguousarray(eye_b).view(np.float32)  # [128, 64]
    in_maps = []
    for b in range(NCORES):
        idm = np.concatenate(
            [np.stack([de_mod[b], bias], axis=1), eye_b_as_f], axis=1
        ).astype(np.float32)
        in_maps.append(
            {
                "x": np.ascontiguousarray(xp[b]),
                "WbT": wbT,
                "LL": ll,
                "IDm": np.ascontiguousarray(idm),
            }
        )
    return in_maps


def run(inputs, trace=False, trace_kwargs=None):
    nc = _get_nc()
    in_maps = _make_in_maps(**inputs)
    res = run_bass_kernel_spmd(
        nc,
        in_maps,
        core_ids=list(range(NCORES)),
        trace=trace,
        **(trace_kwargs or {}),
    )
    y = np.stack(
        [res.results[b]["y"].astype(np.float32) for b in range(NCORES)], axis=0
    )
    return y, res


def kernel(**inputs):
    y, _ = run(inputs)
    return y



# revision 9
# speedup vs baseline: 1.0178x; 1.0178x over previous
"""Trainium2 Bass kernel for nn_NeuronS3DiffUpsample2D.

Reference computation (per sample b):
    up   = nearest-2x-upsample(x[b])                       # [C, 320, 320]
    w    = Wb + 0.25 * einsum('or,rikl->oikl', lora_up, lora_down)
    w_b  = w * de_mod[b, None, :, None, None]              # modulate input chans
    dem  = rsqrt(sum_{i,k,l} w_b^2 + eps)                  # per output chan
    y[b] = conv2d(up, w_b, SAME) * dem + bias

Key algebraic transform: a 3x3 SAME conv on a 2x nearest-upsampled image
decomposes into 4 output phases (di, dj in {0,1}), each a 2x2 conv on the
ORIGINAL 160x160 input:
    y[2i+di, 2j+dj] = sum_{a,b in {0,1}} K[di,dj,a,b] @ x[i+a+di-1, j+b+dj-1]
where the 16 [O, I] matrices K are sums of 1/2/4 of the 9 taps of w.
This is 4/9 of the naive FLOPs and never materializes the upsampled image.
The demod scale is per output channel and conv is linear in w, so the conv
OUTPUT is scaled by dem[o] at PSUM eviction, fused with the bias add.

All tensors ride bf16 (rel-err budget is 2e-2; bf16 end-to-end lands ~4e-3):
x is host-padded to [C,162,162] bf16 so every band DMA is one contiguous
descriptor per partition (no per-row 640B descriptors, no on-device border
zeroing), matmuls are bf16 (1 cycle/row at any free size - also kills the
f32r N<256 4x penalty on the R=1 tail block), and y is written bf16 and
upcast on the host.  Total HBM traffic drops from 66 MB to ~35 MB per core,
taking DMA well below the PE's ~173 us of matmul streaming.

The demod reduction runs in a second [o, (t,i)] weight layout so the
partition-axis sum becomes a free-axis reduce: 3 small early matmuls build
delta in [o,.] form, then gpsimd/DVE/ACT finish rsqrt off the PE's program
order (the baseline's ones-matmul made the PE wait ~10us on a DVE chain).
Dummy "warm" matmuls keep the PE busy through the weight stage so the
2.4 GHz pstate ramp completes before the main conv stream starts.

Sharding: data-parallel over batch B=8 across 8 NeuronCores; each core
builds its own per-sample weights locally.  Host-side work is layout only:
padding, per-sample slicing/replication, transposes, dtype casts.
"""

import os
import sys
import numpy as np
from contextlib import ExitStack

_NOWARM = bool(int(os.environ.get("K_NOWARM", "0")))
_NOGPS = bool(int(os.environ.get("K_NOGPS", "0")))
_NODEMO = bool(int(os.environ.get("K_NODEMO", "0")))   # demod=1.0 (wrong numerics, hang-test only)
_F32OUT = bool(int(os.environ.get("K_F32OUT", "0")))
_NOACTMUL = bool(int(os.environ.get("K_NOACTMUL", "0")))
_DEBUG = bool(int(os.environ.get("K_DEBUG", "0")))

try:
    import concourse.bass as bass
except ImportError:  # grading env without the axon PYTHONPATH
    sys.path.insert(0, "/opt/trn_rl_repo")
    import concourse.bass as bass
import ml_dtypes
import concourse.tile as tile
from concourse import bacc, mybir
from concourse.bass_utils import run_bass_kernel_spmd

B, C, H, W = 8, 128, 160, 160
RANK = 32
SCALING = 0.25
EPS = 1e-8
HP, WP = H + 2, W + 2      # host-padded input (zero border baked in)
R_BLK = 3                  # x-rows per matmul block -> N = 3*160 = 480 <= 512
# x-row band split: band 0 tiny so the first matmuls gate only on ~0.3 MB.
BAND_ROWS = [3, 12, 27, 27, 27, 27, 27, 10]
BAND_START = [0, 3, 15, 42, 69, 96, 123, 150]
NBANDS = len(BAND_ROWS)
NBLOCKS = 54               # 53 blocks of 3 rows + 1 tail row
N_WARM = 8                 # PE HAM-warm matmuls (memset-gated, cover boot->WbTS)
NCORES = 8

f32 = mybir.dt.float32
bf16 = mybir.dt.bfloat16


def _conv_kernel(ctx, tc, y, x, wbT, ll_d, idm_d, dbg=None):
    nc = tc.nc
    AF = mybir.ActivationFunctionType
    ALU = mybir.AluOpType
    AX = mybir.AxisListType

    const = ctx.enter_context(tc.tile_pool(name="const", bufs=1))
    bands = ctx.enter_context(tc.tile_pool(name="bands", bufs=4))

    comb = const.tile([128, 16, C], bf16)        # 16 combined taps, [i, slot, o]
    demP = const.tile([128, 1], f32)             # rsqrt demod, per output chan
    biasT = const.tile([128, 1], f32)            # bias, copied out of IDm

    # ---- input bands: host-padded rows [S, S+rows+2) land contiguously; one
    # descriptor per partition.  Bands 0/1 ride the sync HWDGE ring (boots
    # first), the rest the gpsimd SWDGE ring.
    band_tiles = []
    band_dmas = []
    for bb in range(NBANDS):
        n = BAND_ROWS[bb] + 2
        bt = bands.tile([128, n, WP], bf16, tag="band", name=f"band{bb}")
        band_tiles.append(bt)
        band_dmas.append((bt[:, 0:n, :], x[:, BAND_START[bb] : BAND_START[bb] + n, :]))

    with tc.tile_pool(name="wtmp", bufs=1) as wtmp, tc.tile_pool(
        name="wpsum", bufs=1, space="PSUM"
    ) as wpsum:
        # warm-matmul operand: a memset tile so the PE can start the moment
        # the engines boot (~6.4us) instead of gating on the LL DMA (~10us).
        # The DVFS ramp to 2.4 GHz takes ~15us of sustained PE activity, so
        # every us of earlier PE onset is a us of main-loop time at full
        # clock instead of 1.2 GHz.
        warmT = wtmp.tile([128, 512], bf16, name="warmsrc")
        nc.gpsimd.memset(warmT[:], 0.0)

        LL = wtmp.tile([RANK, 1280], bf16)       # lora_down^T (t,i) | 0.25*lora_up^T
        WbTS = wtmp.tile([128, 9, C], bf16)      # Wb^T: [i, t, o]
        IDm = wtmp.tile([128, 66], f32)          # de_mod,bias | bf16 eye
        LD9 = LL[:, 0:1152].rearrange("p (t c) -> p t c", c=C)
        LUTn = LL[:, 1152:1280]
        dmb = IDm[:, 0:2]
        identb = IDm[:, 2:66].bitcast(bf16)

        # The SWDGE (gpsimd) queue hits full rate (~380 GB/s) immediately,
        # while the HWDGE (sync) queue crawls at <70 GB/s for its first ~8us.
        # So ALL startup tensors ride SWDGE; the BIGGEST tensor (WbTS) goes
        # first so the small ones pipeline behind it on the ring — the PE
        # pairs gate on WbTS, which is the last sem to fire either way.
        # Sync gets a small primer so its ramp is done before the first
        # output eviction.
        nc.gpsimd.dma_start(WbTS[:], wbT[:])
        nc.gpsimd.dma_start(LL[:], ll_d[:])
        nc.gpsimd.dma_start(IDm[:], idm_d[:])
        for bb in range(4):
            nc.gpsimd.dma_start(*band_dmas[bb])
        LLscratch = wtmp.tile([RANK, 1280], bf16, name="llscratch")
        nc.sync.dma_start(LLscratch[:], ll_d[:])

        # ---- PE early: prewarm on the memset tile (no DMA dependency),
        # then per tap an adjacent lora+identity pair accumulating
        #   deltaP[i,t,o] = sum_r down[r,i,t]*0.25*up[o,r] + Wb^T[i,t,o]
        # (folds the base weight in on the PE).  The pairs must be adjacent:
        # interleaving other matmuls inside an open accumulation group drops
        # the first write.
        if not _NOWARM:
            warmP = wpsum.tile([128, 384], f32)
            for _ in range(N_WARM):
                nc.tensor.matmul(
                    warmP[:], warmT[:, 0:128], warmT[:, 128:512],
                    start=True, stop=True,
                )
        deltaP = wpsum.tile([128, 9, C], f32)
        S2P = wpsum.tile([128, 1], f32)
        gps = nc.vector if _NOGPS else nc.gpsimd
        wm3 = wtmp.tile([128, 9, C], bf16)
        Wsq = wtmp.tile([128, 9, C], bf16)
        dm2b = wtmp.tile([128, 1], bf16)

        for t in range(9):
            nc.tensor.matmul(
                deltaP[:, t, :], LD9[:, t, :], LUTn[:], start=True, stop=False
            )
            nc.tensor.matmul(
                deltaP[:, t, :], identb[:], WbTS[:, t, :], start=False, stop=True
            )

        # ---- demod, separable form: demod[o] = rsqrt(sum_i dm[i]^2 *
        # sum_t W[i,t,o]^2 + eps).  ACT squares deltaP 3 taps at a time
        # (chasing the pairs), then 9 tiny accumulating matmuls with Wsq
        # stationary and dm^2 as an N=1 moving operand put the partition
        # reduction DIRECTLY in [o,1] orientation — no PE transposes, no
        # PSUM-resident wmOT blocking the main loop's PSUM reuse.
        # bias is read during evictions long after IDm's SBUF may recycle.
        nc.scalar.copy(biasT[:], dmb[:, 1:2])
        if not _NODEMO:
            for cch in range(3):
                nc.scalar.square(
                    Wsq[:, 3 * cch : 3 * cch + 3, :],
                    deltaP[:, 3 * cch : 3 * cch + 3, :],
                )

        # ---- modulated weights + 16 combined tap matrices.
        # wm3[i,t,o] = deltaP * de_mod[i], computed in 3-tap chunks on DVE
        # chasing the pairs.  Row combos over ki (t = 3*ki + kj):
        #   (di=0, a=0): ki0        (di=0, a=1): ki1+ki2
        #   (di=1, a=0): ki0+ki1    (di=1, a=1): ki2
        # and the same pattern over kj for (dj, b).  Slot ops are emitted
        # interleaved with the wm3 chunks, hand-placed per engine queue so
        # each op's inputs are ready just before the queue reaches it:
        # DVE gets phases 0/3 (+R01), gpsimd phases 1/2 (+R10).
        nc.vector.tensor_mul(dm2b[:], dmb[:, 0:1], dmb[:, 0:1])

        R01 = wtmp.tile([128, 3, C], bf16)
        R10 = wtmp.tile([128, 3, C], bf16)
        rowsrc = {
            (0, 0): wm3[:, 0:3, :],
            (0, 1): R01[:],
            (1, 0): R10[:],
            (1, 1): wm3[:, 6:9, :],
        }

        # comb slot layout: slot = 8*di + 2*a + 4*dj + b (bf16 on write).
        def build_slot(eng, p, q):
            di, dj = p >> 1, p & 1
            a, b = q >> 1, q & 1
            S = rowsrc[(di, a)]
            dst = comb[:, 8 * di + 2 * a + 4 * dj + b, :]
            if dj == 0 and b == 0:
                eng.tensor_copy(dst, S[:, 0, :])
            elif dj == 1 and b == 1:
                eng.tensor_copy(dst, S[:, 2, :])
            elif dj == 0:
                eng.tensor_add(dst, S[:, 1, :], S[:, 2, :])
            else:
                eng.tensor_add(dst, S[:, 0, :], S[:, 1, :])

        def wm3_chunk(cch):
            nc.vector.tensor_scalar_mul(
                wm3[:, 3 * cch : 3 * cch + 3, :],
                deltaP[:, 3 * cch : 3 * cch + 3, :],
                dmb[:, 0:1],
            )

        wm3_chunk(0)
        build_slot(nc.vector, 0, 0)      # copy wm3[0]
        build_slot(gps, 1, 0)            # add  wm3[0]+wm3[1]
        build_slot(nc.vector, 0, 1)      # add  wm3[1]+wm3[2]
        build_slot(gps, 1, 1)            # copy wm3[2]
        wm3_chunk(1)
        gps.tensor_add(R10[:], wm3[:, 0:3, :], wm3[:, 3:6, :])
        wm3_chunk(2)
        nc.vector.tensor_add(R01[:], wm3[:, 3:6, :], wm3[:, 6:9, :])
        build_slot(nc.vector, 0, 2)      # copy R01[0]
        build_slot(nc.vector, 0, 3)      # add  R01[1]+R01[2]
        build_slot(gps, 1, 2)            # add  R01[0]+R01[1]
        build_slot(gps, 1, 3)            # copy R01[2]
        for q in range(4):
            build_slot(gps, 2, q)        # R10 / wm3[6:9] col-combines

        # demod partition-reduction matmuls (PE, right after the pairs) +
        # eps/sqrt/reciprocal tail.  S2P[o] = sum_t sum_i Wsq[i,t,o]*dm2[i].
        if _NODEMO:
            nc.vector.memset(demP[:], 1.0)
        else:
            for t in range(9):
                nc.tensor.matmul(
                    S2P[:], Wsq[:, t, :], dm2b[:],
                    start=(t == 0), stop=(t == 8),
                )
            t2 = wtmp.tile([128, 1], f32)
            nc.scalar.activation(t2[:], S2P[:], AF.Sqrt, bias=EPS)
            nc.vector.reciprocal(demP[:], t2[:])

        for q in range(4):
            build_slot(nc.vector, 3, q)  # R10 / wm3[6:9] col-combines

        if dbg is not None:
            nc.sync.dma_start(dbg["wm3"][:], wm3[:])
            nc.sync.dma_start(dbg["comb"][:], comb.rearrange("p s c -> p (s c)"))
            nc.sync.dma_start(dbg["demP"][:], demP[:])

        # now it is safe to park gpsimd on the recycled-band WAR semaphores
        # (bands 4+ reuse band 0-3 buffers)
        for bb in range(4, NBANDS):
            nc.gpsimd.dma_start(*band_dmas[bb])

    # ---- main conv loop ----
    mpsum = ctx.enter_context(tc.tile_pool(name="mpsum", bufs=8, space="PSUM"))
    opool = ctx.enter_context(tc.tile_pool(name="obuf", bufs=4))

    # blocks of 3 x-rows; pairs of blocks share one output staging tile so
    # the sync sequencer issues half as many (128-descriptor) output DMAs.
    blk_band = []
    for g in range(NBLOCKS):
        i0 = g * R_BLK
        bb = next(
            b for b in range(NBANDS)
            if BAND_START[b] <= i0 and i0 + min(R_BLK, H - i0) <= BAND_START[b] + BAND_ROWS[b]
        )
        blk_band.append(bb)

    ob = None
    for g in range(NBLOCKS):
        i0 = g * R_BLK
        R = min(R_BLK, H - i0)
        bb = blk_band[g]
        bt = band_tiles[bb]
        ph = []
        for p in range(4):
            di, dj = p >> 1, p & 1
            pt = mpsum.tile([128, R * W], f32, tag="ph", name=f"ph{p}_{i0}")
            for q in range(4):
                a, b = q >> 1, q & 1
                tr = i0 + a + di - BAND_START[bb]     # padded-tile row
                co = b + dj                           # padded-tile col
                rhs = bt[:, tr : tr + R, co : co + W]
                slot = 8 * di + 2 * a + 4 * dj + b
                nc.tensor.matmul(
                    pt[:], comb[:, slot, :], rhs,
                    start=(q == 0), stop=(q == 3),
                )
            ph.append(pt)

        if g % 2 == 0:
            RR = R + (min(R_BLK, H - i0 - R) if g + 1 < NBLOCKS else 0)
            ob = opool.tile(
                [128, RR, 2, 2 * W], f32 if _F32OUT else bf16, tag="ob", name=f"ob_{i0}"
            )
            lr0 = 0
        else:
            lr0 = R_BLK
        obv = ob.rearrange("p r d (j two) -> p r d two j", two=2)
        # interleave phases into full output rows; scale by demod, add bias
        for p in range(4):
            di, dj = p >> 1, p & 1
            dst = obv[:, lr0 : lr0 + R, di, dj, :]
            srcv = ph[p].rearrange("p (r j) -> p r j", r=R)
            if dj == 0:
                nc.vector.tensor_scalar(
                    dst, srcv, demP[:, 0:1], biasT[:, 0:1],
                    op0=ALU.mult, op1=ALU.add,
                )
            else:
                nc.scalar.activation(
                    dst, srcv, AF.Identity, bias=biasT[:, 0:1], scale=demP[:, 0:1]
                )
        if g % 2 == 1 or g == NBLOCKS - 1:
            g0 = g - (g % 2)
            y0 = 2 * g0 * R_BLK
            nrows = 2 * ob.shape[1]
            if g == NBLOCKS - 1:
                # final group: split across BOTH DMA rings so the end-of-
                # kernel drain (which gates the teardown barrier) halves.
                rh = ob.shape[1] // 2
                nc.sync.dma_start(y[:, y0 : y0 + 2 * rh, :], ob[:, 0:rh])
                nc.gpsimd.dma_start(y[:, y0 + 2 * rh : y0 + nrows, :], ob[:, rh:])
            else:
                eng = nc.sync if (g0 // 2) % 2 == 0 else nc.gpsimd
                eng.dma_start(y[:, y0 : y0 + nrows, :], ob[:])


def _build():
    nc = bacc.Bacc(
        "TRN2",
        target_bir_lowering=False,
        debug=False,
        enable_asserts=False,
        num_devices=NCORES,
    )
    x = nc.dram_tensor("x", [C, HP, WP], bf16, kind="ExternalInput").ap()
    wbT = nc.dram_tensor("WbT", [C, 9 * C], bf16, kind="ExternalInput").ap()
    ll_d = nc.dram_tensor("LL", [RANK, 1280], bf16, kind="ExternalInput").ap()
    idm_d = nc.dram_tensor("IDm", [128, 66], f32, kind="ExternalInput").ap()
    y = nc.dram_tensor(
        "y", [C, 2 * H, 2 * W], f32 if _F32OUT else bf16, kind="ExternalOutput"
    ).ap()
    dbg = None
    if _DEBUG:
        dbg = {
            "wm3": nc.dram_tensor("dbg_wm3", [C, 9 * C], f32, kind="ExternalOutput").ap(),
            "comb": nc.dram_tensor("dbg_comb", [C, 16 * C], bf16, kind="ExternalOutput").ap(),
            "demP": nc.dram_tensor("dbg_demP", [C, 1], f32, kind="ExternalOutput").ap(),
        }

    with tile.TileContext(nc) as tc:
        with ExitStack() as ctx:
            _conv_kernel(ctx, tc, y, x, wbT, ll_d, idm_d, dbg)
    nc.compile()
    return nc


_CACHE = {}


def _get_nc():
    if "nc" not in _CACHE:
        _CACHE["nc"] = _build()
    return _CACHE["nc"]


def _make_in_maps(x, de_mod, Wb, lora_up, lora_down, bias):
    x = np.asarray(x, dtype=np.float32)
    de_mod = np.asarray(de_mod, dtype=np.float32)
    Wb = np.asarray(Wb, dtype=np.float32)
    lora_up = np.asarray(lora_up, dtype=np.float32)
    lora_down = np.asarray(lora_down, dtype=np.float32)
    bias = np.asarray(bias, dtype=np.float32).reshape(C)
    # layout-only host prep: pad + cast x, transpose/replicate weights
    xp = np.zeros((B, C, HP, WP), dtype=ml_dtypes.bfloat16)
    xp[:, :, 1 : 1 + H, 1 : 1 + W] = x.astype(ml_dtypes.bfloat16)
    wbT = np.ascontiguousarray(
        Wb.transpose(1, 2, 3, 0).reshape(C, 9 * C)
    ).astype(ml_dtypes.bfloat16)
    luT = np.ascontiguousarray((SCALING * lora_up).T).astype(ml_dtypes.bfloat16)
    ldT = np.ascontiguousarray(
        lora_down.transpose(0, 2, 3, 1).reshape(RANK, 9 * C)
    ).astype(ml_dtypes.bfloat16)
    ll = np.concatenate([ldT, luT], axis=1)      # [32, 1280] bf16
    eye_b = np.eye(128, dtype=ml_dtypes.bfloat16)
    eye_b_as_f = np.ascontiguousarray(eye_b).view(np.float32)  # [128, 64]
    in_maps = []
    for b in range(NCORES):
        idm = np.concatenate(
            [np.stack([de_mod[b], bias], axis=1), eye_b_as_f], axis=1
        ).astype(np.float32)
        in_maps.append(
            {
                "x": np.ascontiguousarray(xp[b]),
                "WbT": wbT,
                "LL": ll,
                "IDm": np.ascontiguousarray(idm),
            }
        )
    return in_maps


def run(inputs, trace=False, trace_kwargs=None):
    nc = _get_nc()
    in_maps = _make_in_maps(**inputs)
    res = run_bass_kernel_spmd(
        nc,
        in_maps,
        core_ids=list(range(NCORES)),
        trace=trace,
        **(trace_kwargs or {}),
    )
    y = np.stack(
        [res.results[b]["y"].astype(np.float32) for b in range(NCORES)], axis=0
    )
    return y, res


def kernel(**inputs):
    y, _ = run(inputs)
    return y



# revision 17
# speedup vs baseline: 1.0410x; 1.0228x over previous
"""Trainium2 Bass kernel for nn_NeuronS3DiffUpsample2D.

Reference computation (per sample b):
    up   = nearest-2x-upsample(x[b])                       # [C, 320, 320]
    w    = Wb + 0.25 * einsum('or,rikl->oikl', lora_up, lora_down)
    w_b  = w * de_mod[b, None, :, None, None]              # modulate input chans
    dem  = rsqrt(sum_{i,k,l} w_b^2 + eps)                  # per output chan
    y[b] = conv2d(up, w_b, SAME) * dem + bias

Key algebraic transform: a 3x3 SAME conv on a 2x nearest-upsampled image
decomposes into 4 output phases (di, dj in {0,1}), each a 2x2 conv on the
ORIGINAL 160x160 input:
    y[2i+di, 2j+dj] = sum_{a,b in {0,1}} K[di,dj,a,b] @ x[i+a+di-1, j+b+dj-1]
where the 16 [O, I] matrices K are sums of 1/2/4 of the 9 taps of w.
This is 4/9 of the naive FLOPs and never materializes the upsampled image.
The demod scale is per output channel and conv is linear in w, so the conv
OUTPUT is scaled by dem[o] at PSUM eviction, fused with the bias add.

All tensors ride bf16 (rel-err budget is 2e-2; bf16 end-to-end lands ~4e-3):
x is host-padded to [C,162,162] bf16 so every band DMA is one contiguous
descriptor per partition (no per-row 640B descriptors, no on-device border
zeroing), matmuls are bf16 (1 cycle/row at any free size - also kills the
f32r N<256 4x penalty on the R=1 tail block), and y is written bf16 and
upcast on the host.  Total HBM traffic drops from 66 MB to ~35 MB per core,
taking DMA well below the PE's ~173 us of matmul streaming.

The demod reduction runs in a second [o, (t,i)] weight layout so the
partition-axis sum becomes a free-axis reduce: 3 small early matmuls build
delta in [o,.] form, then gpsimd/DVE/ACT finish rsqrt off the PE's program
order (the baseline's ones-matmul made the PE wait ~10us on a DVE chain).
Dummy "warm" matmuls keep the PE busy through the weight stage so the
2.4 GHz pstate ramp completes before the main conv stream starts.

Sharding: data-parallel over batch B=8 across 8 NeuronCores; each core
builds its own per-sample weights locally.  Host-side work is layout only:
padding, per-sample slicing/replication, transposes, dtype casts.
"""

import os
import sys
import numpy as np
from contextlib import ExitStack

_NOWARM = bool(int(os.environ.get("K_NOWARM", "0")))
_NOGPS = bool(int(os.environ.get("K_NOGPS", "0")))
_NODEMO = bool(int(os.environ.get("K_NODEMO", "0")))   # demod=1.0 (wrong numerics, hang-test only)
_F32OUT = bool(int(os.environ.get("K_F32OUT", "0")))
_NOACTMUL = bool(int(os.environ.get("K_NOACTMUL", "0")))
_DEBUG = bool(int(os.environ.get("K_DEBUG", "0")))

try:
    import concourse.bass as bass
except ImportError:  # grading env without the axon PYTHONPATH
    sys.path.insert(0, "/opt/trn_rl_repo")
    import concourse.bass as bass
import ml_dtypes
import concourse.tile as tile
from concourse import bacc, mybir
from concourse.bass_utils import run_bass_kernel_spmd

B, C, H, W = 8, 128, 160, 160
RANK = 32
SCALING = 0.25
EPS = 1e-8
HP, WP = H + 2, W + 2      # host-padded input (zero border baked in)
R_BLK = 3                  # x-rows per matmul block -> N = 3*160 = 480 <= 512
# x-row band split: band 0 tiny so the first matmuls gate only on ~0.3 MB.
BAND_ROWS = [3, 12, 27, 27, 27, 27, 27, 10]
BAND_START = [0, 3, 15, 42, 69, 96, 123, 150]
NBANDS = len(BAND_ROWS)
NBLOCKS = 54               # 53 blocks of 3 rows + 1 tail row
N_WARM = 12                # PE HAM-warm matmuls (memset-gated, cover boot->WbTS)
NCORES = 8

f32 = mybir.dt.float32
bf16 = mybir.dt.bfloat16


def _conv_kernel(ctx, tc, y, x, wbT, ll_d, idm_d, dbg=None):
    nc = tc.nc
    AF = mybir.ActivationFunctionType
    ALU = mybir.AluOpType
    AX = mybir.AxisListType

    const = ctx.enter_context(tc.tile_pool(name="const", bufs=1))
    bands = ctx.enter_context(tc.tile_pool(name="bands", bufs=4))

    comb = const.tile([128, 16, C], bf16)        # 16 combined taps, [i, slot, o]
    demP = const.tile([128, 1], f32)             # rsqrt demod, per output chan
    biasT = const.tile([128, 1], f32)            # bias, copied out of IDm

    # ---- input bands: host-padded rows [S, S+rows+2) land contiguously; one
    # descriptor per partition.  Bands 0/1 ride the sync HWDGE ring (boots
    # first), the rest the gpsimd SWDGE ring.
    band_tiles = []
    band_dmas = []
    for bb in range(NBANDS):
        n = BAND_ROWS[bb] + 2
        bt = bands.tile([128, n, WP], bf16, tag="band", name=f"band{bb}")
        band_tiles.append(bt)
        band_dmas.append((bt[:, 0:n, :], x[:, BAND_START[bb] : BAND_START[bb] + n, :]))

    with tc.tile_pool(name="wtmp", bufs=1) as wtmp, tc.tile_pool(
        name="wpsum", bufs=1, space="PSUM"
    ) as wpsum:
        # warm-matmul operand: a memset tile so the PE can start the moment
        # the engines boot (~6.4us) instead of gating on the LL DMA (~10us).
        # The DVFS ramp to 2.4 GHz takes ~15us of sustained PE activity, so
        # every us of earlier PE onset is a us of main-loop time at full
        # clock instead of 1.2 GHz.
        warmT = wtmp.tile([128, 512], bf16, name="warmsrc")
        nc.gpsimd.memset(warmT[:], 0.0)
        # dummy sqrt: forces the framework's ACT_TABLE_LOAD (sqrt table,
        # ~1.3us) to land here while ACT is idle, not right before the
        # demod sqrt on the critical path.
        sqwarm = wtmp.tile([128, 1], f32, name="sqwarm")
        nc.scalar.sqrt(sqwarm[:], warmT[:, 0:1])
        epsT = wtmp.tile([128, 1], f32, name="epsT")
        nc.gpsimd.memset(epsT[:], EPS)

        LL = wtmp.tile([RANK, 1280], bf16)       # lora_down^T (t,i) | 0.25*lora_up^T
        WbTS = wtmp.tile([128, 9, C], bf16)      # Wb^T: [i, t, o]
        IDm = wtmp.tile([128, 66], f32)          # de_mod,bias | bf16 eye
        LD9 = LL[:, 0:1152].rearrange("p (t c) -> p t c", c=C)
        LUTn = LL[:, 1152:1280]
        dmb = IDm[:, 0:2]
        identb = IDm[:, 2:66].bitcast(bf16)

        # The SWDGE (gpsimd) queue hits full rate (~380 GB/s) immediately,
        # while the HWDGE (sync) queue crawls at <70 GB/s for its first ~8us.
        # So ALL startup tensors ride SWDGE; the BIGGEST tensor (WbTS) goes
        # first so the small ones pipeline behind it on the ring — the PE
        # pairs gate on WbTS, which is the last sem to fire either way.
        # Sync gets a small primer so its ramp is done before the first
        # output eviction.
        nc.gpsimd.dma_start(WbTS[:], wbT[:])
        nc.gpsimd.dma_start(LL[:], ll_d[:])
        nc.gpsimd.dma_start(IDm[:], idm_d[:])
        for bb in range(4):
            nc.gpsimd.dma_start(*band_dmas[bb])
        LLscratch = wtmp.tile([RANK, 1280], bf16, name="llscratch")
        nc.sync.dma_start(LLscratch[:], ll_d[:])

        # ---- PE early: prewarm on the memset tile (no DMA dependency),
        # then per tap an adjacent lora+identity pair accumulating
        #   deltaP[i,t,o] = sum_r down[r,i,t]*0.25*up[o,r] + Wb^T[i,t,o]
        # (folds the base weight in on the PE).  The pairs must be adjacent:
        # interleaving other matmuls inside an open accumulation group drops
        # the first write.
        if not _NOWARM:
            warmP = wpsum.tile([128, 384], f32)
            for _ in range(N_WARM):
                nc.tensor.matmul(
                    warmP[:], warmT[:, 0:128], warmT[:, 128:512],
                    start=True, stop=True,
                )
        # deltaP is split into three 3-tap PSUM tiles: Tile's dependency
        # tracking is whole-tile for PSUM, so with a single [128,9,C] tile
        # every downstream chunk (Wsq squares, wm3 de_mod muls) would gate
        # on ALL 18 pair matmuls instead of chasing them 3 taps at a time,
        # and the PSUM banks would stay WAR-busy until the very last read.
        deltaPs = [wpsum.tile([128, 3, C], f32, name=f"deltaP{k}") for k in range(3)]
        S2P = wpsum.tile([128, 1], f32)
        gps = nc.vector if _NOGPS else nc.gpsimd
        wm3 = wtmp.tile([128, 9, C], bf16)
        Wsq = wtmp.tile([128, 9, C], bf16)
        dm2b = wtmp.tile([128, 1], bf16)

        def deltaP(t):
            return deltaPs[t // 3][:, t % 3, :]

        for t in range(9):
            nc.tensor.matmul(
                deltaP(t), LD9[:, t, :], LUTn[:], start=True, stop=False
            )
            nc.tensor.matmul(
                deltaP(t), identb[:], WbTS[:, t, :], start=False, stop=True
            )

        # ---- demod, separable form: demod[o] = rsqrt(sum_i dm[i]^2 *
        # sum_t W[i,t,o]^2 + eps).  ACT squares deltaP 3 taps at a time
        # (chasing the pairs), then 9 tiny accumulating matmuls with Wsq
        # stationary and dm^2 as an N=1 moving operand put the partition
        # reduction DIRECTLY in [o,1] orientation — no PE transposes, no
        # PSUM-resident wmOT blocking the main loop's PSUM reuse.
        # bias is read during evictions long after IDm's SBUF may recycle.
        nc.scalar.copy(biasT[:], dmb[:, 1:2])
        if not _NODEMO:
            for cch in range(3):
                nc.scalar.square(
                    Wsq[:, 3 * cch : 3 * cch + 3, :], deltaPs[cch][:]
                )

        # ---- modulated weights + 16 combined tap matrices.
        # wm3[i,t,o] = deltaP * de_mod[i], computed in 3-tap chunks on DVE
        # chasing the pairs.  Row combos over ki (t = 3*ki + kj):
        #   (di=0, a=0): ki0        (di=0, a=1): ki1+ki2
        #   (di=1, a=0): ki0+ki1    (di=1, a=1): ki2
        # and the same pattern over kj for (dj, b).  Slot ops are emitted
        # interleaved with the wm3 chunks, hand-placed per engine queue so
        # each op's inputs are ready just before the queue reaches it:
        # DVE gets phases 0/3 (+R01), gpsimd phases 1/2 (+R10).
        nc.vector.tensor_mul(dm2b[:], dmb[:, 0:1], dmb[:, 0:1])

        R01 = wtmp.tile([128, 3, C], bf16)
        R10 = wtmp.tile([128, 3, C], bf16)
        rowsrc = {
            (0, 0): wm3[:, 0:3, :],
            (0, 1): R01[:],
            (1, 0): R10[:],
            (1, 1): wm3[:, 6:9, :],
        }

        # comb slot layout: slot = 8*di + 2*a + 4*dj + b (bf16 on write).
        def build_slot(eng, p, q):
            di, dj = p >> 1, p & 1
            a, b = q >> 1, q & 1
            S = rowsrc[(di, a)]
            dst = comb[:, 8 * di + 2 * a + 4 * dj + b, :]
            if dj == 0 and b == 0:
                eng.tensor_copy(dst, S[:, 0, :])
            elif dj == 1 and b == 1:
                eng.tensor_copy(dst, S[:, 2, :])
            elif dj == 0:
                eng.tensor_add(dst, S[:, 1, :], S[:, 2, :])
            else:
                eng.tensor_add(dst, S[:, 0, :], S[:, 1, :])

        def wm3_chunk(cch):
            nc.vector.tensor_scalar_mul(
                wm3[:, 3 * cch : 3 * cch + 3, :], deltaPs[cch][:], dmb[:, 0:1]
            )

        wm3_chunk(0)
        build_slot(nc.vector, 0, 0)      # copy wm3[0]
        build_slot(gps, 1, 0)            # add  wm3[0]+wm3[1]
        build_slot(nc.vector, 0, 1)      # add  wm3[1]+wm3[2]
        build_slot(gps, 1, 1)            # copy wm3[2]
        wm3_chunk(1)
        gps.tensor_add(R10[:], wm3[:, 0:3, :], wm3[:, 3:6, :])
        wm3_chunk(2)
        nc.vector.tensor_add(R01[:], wm3[:, 3:6, :], wm3[:, 6:9, :])
        build_slot(nc.vector, 0, 2)      # copy R01[0]
        build_slot(nc.vector, 0, 3)      # add  R01[1]+R01[2]
        build_slot(gps, 1, 2)            # add  R01[0]+R01[1]
        build_slot(gps, 1, 3)            # copy R01[2]
        for q in range(4):
            build_slot(gps, 2, q)        # R10 / wm3[6:9] col-combines

        # demod partition-reduction matmuls (PE, right after the pairs) +
        # eps/sqrt/reciprocal tail.  S2P[o] = sum_t sum_i Wsq[i,t,o]*dm2[i].
        if _NODEMO:
            nc.vector.memset(demP[:], 1.0)
        else:
            for t in range(9):
                nc.tensor.matmul(
                    S2P[:], Wsq[:, t, :], dm2b[:],
                    start=(t == 0), stop=(t == 8),
                )
            t2 = wtmp.tile([128, 1], f32)
            nc.scalar.activation(t2[:], S2P[:], AF.Sqrt, bias=epsT[:, 0:1])
            nc.vector.reciprocal(demP[:], t2[:])

        for q in range(4):
            build_slot(nc.vector, 3, q)  # R10 / wm3[6:9] col-combines

        if dbg is not None:
            nc.sync.dma_start(dbg["wm3"][:], wm3[:])
            nc.sync.dma_start(dbg["comb"][:], comb.rearrange("p s c -> p (s c)"))
            nc.sync.dma_start(dbg["demP"][:], demP[:])

        # now it is safe to park gpsimd on the recycled-band WAR semaphores
        # (bands 4+ reuse band 0-3 buffers)
        for bb in range(4, NBANDS):
            nc.gpsimd.dma_start(*band_dmas[bb])

    # ---- main conv loop ----
    mpsum = ctx.enter_context(tc.tile_pool(name="mpsum", bufs=8, space="PSUM"))
    opool = ctx.enter_context(tc.tile_pool(name="obuf", bufs=4))

    # blocks of 3 x-rows; pairs of blocks share one output staging tile so
    # the sync sequencer issues half as many (128-descriptor) output DMAs.
    blk_band = []
    for g in range(NBLOCKS):
        i0 = g * R_BLK
        bb = next(
            b for b in range(NBANDS)
            if BAND_START[b] <= i0 and i0 + min(R_BLK, H - i0) <= BAND_START[b] + BAND_ROWS[b]
        )
        blk_band.append(bb)

    ob = None
    for g in range(NBLOCKS):
        i0 = g * R_BLK
        R = min(R_BLK, H - i0)
        bb = blk_band[g]
        bt = band_tiles[bb]
        ph = []
        for p in range(4):
            di, dj = p >> 1, p & 1
            pt = mpsum.tile([128, R * W], f32, tag="ph", name=f"ph{p}_{i0}")
            for q in range(4):
                a, b = q >> 1, q & 1
                tr = i0 + a + di - BAND_START[bb]     # padded-tile row
                co = b + dj                           # padded-tile col
                rhs = bt[:, tr : tr + R, co : co + W]
                slot = 8 * di + 2 * a + 4 * dj + b
                nc.tensor.matmul(
                    pt[:], comb[:, slot, :], rhs,
                    start=(q == 0), stop=(q == 3),
                )
            ph.append(pt)

        if g % 2 == 0:
            RR = R + (min(R_BLK, H - i0 - R) if g + 1 < NBLOCKS else 0)
            ob = opool.tile(
                [128, RR, 2, 2 * W], f32 if _F32OUT else bf16, tag="ob", name=f"ob_{i0}"
            )
            lr0 = 0
        else:
            lr0 = R_BLK
        obv = ob.rearrange("p r d (j two) -> p r d two j", two=2)
        # interleave phases into full output rows; scale by demod, add bias
        for p in range(4):
            di, dj = p >> 1, p & 1
            dst = obv[:, lr0 : lr0 + R, di, dj, :]
            srcv = ph[p].rearrange("p (r j) -> p r j", r=R)
            if dj == 0:
                nc.vector.tensor_scalar(
                    dst, srcv, demP[:, 0:1], biasT[:, 0:1],
                    op0=ALU.mult, op1=ALU.add,
                )
            else:
                nc.scalar.activation(
                    dst, srcv, AF.Identity, bias=biasT[:, 0:1], scale=demP[:, 0:1]
                )
        if g % 2 == 1 or g == NBLOCKS - 1:
            g0 = g - (g % 2)
            y0 = 2 * g0 * R_BLK
            nrows = 2 * ob.shape[1]
            if g0 >= NBLOCKS - 6:
                # last three groups: split each across BOTH DMA rings so
                # the end-of-kernel drain (which gates the teardown
                # barrier + semaphore-reset epilogue) is halved.
                rh = ob.shape[1] // 2
                nc.sync.dma_start(y[:, y0 : y0 + 2 * rh, :], ob[:, 0:rh])
                nc.gpsimd.dma_start(y[:, y0 + 2 * rh : y0 + nrows, :], ob[:, rh:])
            else:
                eng = nc.sync if (g0 // 2) % 2 == 0 else nc.gpsimd
                eng.dma_start(y[:, y0 : y0 + nrows, :], ob[:])


def _build():
    nc = bacc.Bacc(
        "TRN2",
        target_bir_lowering=False,
        debug=False,
        enable_asserts=False,
        num_devices=NCORES,
    )
    x = nc.dram_tensor("x", [C, HP, WP], bf16, kind="ExternalInput").ap()
    wbT = nc.dram_tensor("WbT", [C, 9 * C], bf16, kind="ExternalInput").ap()
    ll_d = nc.dram_tensor("LL", [RANK, 1280], bf16, kind="ExternalInput").ap()
    idm_d = nc.dram_tensor("IDm", [128, 66], f32, kind="ExternalInput").ap()
    y = nc.dram_tensor(
        "y", [C, 2 * H, 2 * W], f32 if _F32OUT else bf16, kind="ExternalOutput"
    ).ap()
    dbg = None
    if _DEBUG:
        dbg = {
            "wm3": nc.dram_tensor("dbg_wm3", [C, 9 * C], f32, kind="ExternalOutput").ap(),
            "comb": nc.dram_tensor("dbg_comb", [C, 16 * C], bf16, kind="ExternalOutput").ap(),
            "demP": nc.dram_tensor("dbg_demP", [C, 1], f32, kind="ExternalOutput").ap(),
        }

    with tile.TileContext(nc) as tc:
        with ExitStack() as ctx:
            _conv_kernel(ctx, tc, y, x, wbT, ll_d, idm_d, dbg)
    nc.compile()
    return nc


_CACHE = {}


def _get_nc():
    if "nc" not in _CACHE:
        _CACHE["nc"] = _build()
    return _CACHE["nc"]


def _make_in_maps(x, de_mod, Wb, lora_up, lora_down, bias):
    x = np.asarray(x, dtype=np.float32)
    de_mod = np.asarray(de_mod, dtype=np.float32)
    Wb = np.asarray(Wb, dtype=np.float32)
    lora_up = np.asarray(lora_up, dtype=np.float32)
    lora_down = np.asarray(lora_down, dtype=np.float32)
    bias = np.asarray(bias, dtype=np.float32).reshape(C)
    # layout-only host prep: pad + cast x, transpose/replicate weights
    xp = np.zeros((B, C, HP, WP), dtype=ml_dtypes.bfloat16)
    xp[:, :, 1 : 1 + H, 1 : 1 + W] = x.astype(ml_dtypes.bfloat16)
    wbT = np.ascontiguousarray(
        Wb.transpose(1, 2, 3, 0).reshape(C, 9 * C)
    ).astype(ml_dtypes.bfloat16)
    luT = np.ascontiguousarray((SCALING * lora_up).T).astype(ml_dtypes.bfloat16)
    ldT = np.ascontiguousarray(
        lora_down.transpose(0, 2, 3, 1).reshape(RANK, 9 * C)
    ).astype(ml_dtypes.bfloat16)
    ll = np.concatenate([ldT, luT], axis=1)      # [32, 1280] bf16
    eye_b = np.eye(128, dtype=ml_dtypes.bfloat16)
    eye_b_as_f = np.ascontiguousarray(eye_b).view(np.float32)  # [128, 64]
    in_maps = []
    for b in range(NCORES):
        idm = np.concatenate(
            [np.stack([de_mod[b], bias], axis=1), eye_b_as_f], axis=1
        ).astype(np.float32)
        in_maps.append(
            {
                "x": np.ascontiguousarray(xp[b]),
                "WbT": wbT,
                "LL": ll,
                "IDm": np.ascontiguousarray(idm),
            }
        )
    return in_maps


def run(inputs, trace=False, trace_kwargs=None):
    nc = _get_nc()
    in_maps = _make_in_maps(**inputs)
    res = run_bass_kernel_spmd(
        nc,
        in_maps,
        core_ids=list(range(NCORES)),
        trace=trace,
        **(trace_kwargs or {}),
    )
    y = np.stack(
        [res.results[b]["y"].astype(np.float32) for b in range(NCORES)], axis=0
    )
    return y, res


def kernel(**inputs):
    y, _ = run(inputs)
    return y

